# revision 39
# baseline (speedup 1.0000x reference)
"""Trainium2 Bass kernel for nn_Digital_update (dense_mlp), 8 NeuronCores.

Per batch element b, user u:
    B_norm[b,u,:] = sum over 64 antennas of B[b,:,u,:]          # [.., 62]
    x = concat([D[b,u,:], B_norm[b,u,:]])                       # [64]
    h = relu(x@W1+b1); h = relu(h@W2+b2); h = relu(h@W3+b3)
    D1 = sigmoid(h@W4+b4)                                       # [2]
    out[b,u,:] = P * D1 / sum_u(D1)

Design (pure data parallel, 64 batches/core, 4 groups of 16 batches):

* B dominates (64MB/core fp32).  It is host-quantized to fp8e4 with
  error diffusion along the antenna axis (the 64-way sum then matches
  fp32 to ~1 ULP), quartering HBM bytes vs fp32.  Loaded as 4-batch
  "quad" tiles [128, 2, 1984] split across the sync and gpsimd DGE
  rings (a ring keeps only ~1 DMA in flight, so two rings pipeline
  issue with transfer).
* The 64-antenna reduction runs on the TensorEngine in fp8 DoubleRow
  mode (0.5 cycles/row, contraction 256 = 4 batches x 64 antennas per
  pass) against a block-diagonal ones mask; all 4 quads of a group
  PSUM-accumulate into the same [16 x 496] chunks.
* Activations stay feature-major ([feat, rows]) so the 4-layer MLP
  chains with no inter-layer transposes; x^T is assembled via
  per-user-pair PE transposes.  The D features ride bn64's pad columns
  62:64 through the transpose, so there is no separate D writer on x^T.
* fp16 MLP (fp8 fails the 2e-2 gate: ~1.8% RMS per fp8 layer).  PSUM
  stays fp32.  End-to-end max rel error vs fp32 reference ~2.9e-3.
* The PE clock ramps 0.65 -> 2.4 GHz over ~3us of continuous execution
  and resets on stalls, so the schedule fights gaps: warmup matmuls on
  a zeroed tile burn the ramp while the first B quad is in flight;
  the next group's reduction passes are dripped after each MLP stage
  (PE executes its stream in order, so emission order is the schedule);
  each group's sigmoid+normalize is deferred until after the NEXT
  group's x^T assembly so the DVE never blocks the PE at a boundary.
* Weights are host-prepacked to their SBUF layouts (contiguous DMAs -
  the on-the-fly rearranges emitted 512 tiny descriptors and hogged
  the ring ~7us); w2/w4 ride the scalar ring, w3 slots into the gpsimd
  ring between B quads; stores ride the sync ring after the B stream.
"""

import sys

import numpy as np

# concourse (Bass/Tile) lives in the TRN RL repo; make sure it's importable
# even when kernel.py is invoked from a bare directory.
try:
    import concourse  # noqa: F401
except ImportError:
    for _p in ('/opt/trn_rl_repo', '/root/.axon_site/_ro/trn_rl_repo'):
        if _p not in sys.path:
            sys.path.insert(0, _p)
    import concourse  # noqa: F401

N_CORES = 8
BATCH, NUM_M, NUM_USER, FEAT_B = 512, 64, 32, 62
BPC = BATCH // N_CORES            # batches per core = 64
GROUP_B = 16                      # batches per group
GROUPS = BPC // GROUP_B           # 4 groups per core
ROWS_G = GROUP_B * NUM_USER       # 512 rows per group
PAIRS = GROUP_B // 2              # 8 B pair-tiles (2 batches x 64 ants) per group
UF = NUM_USER * FEAT_B            # 1984 contiguous (user, feat) elements
NCHUNK = 4                        # 1984 = 4 x 496 matmul column chunks

# 'fp32'  : exact everywhere (slow)
# 'fp32r' : reduced-precision matmuls everywhere (~11 mantissa bits)
# 'bf16'  : f32r antenna-reduction + bf16 MLP
# 'bf16b' : all-bf16 (B host-cast to bf16 -> half the DMA bytes)
# 'hybrid': fp16 B/reduction (half DMA) + f32r MLP
# 'fp16'  : all-fp16 (full-rate PE, half DMA, ~10 mantissa bits)
# 'fp8'   : fp8e4 B + DoubleRow antenna-reduction (quarter DMA, 2x PE rate,
#           4 batches per reduction pass) + fp16 MLP.  B is quantized with
#           error diffusion along the antenna axis so the 64-way sum stays
#           accurate to ~1 fp8 ULP (max rel err ~2e-3, same as fp16).
PRECISION = 'fp8'

_CACHE = {}


def _build(precision):
    import concourse.bacc as bacc
    import concourse.tile as tile
    from concourse import mybir
    from concourse.bass import ts

    f32 = mybir.dt.float32
    f32r = mybir.dt.float32r
    bf16 = mybir.dt.bfloat16
    f16 = mybir.dt.float16
    f8 = mybir.dt.float8e4
    AF = mybir.ActivationFunctionType
    ALU = mybir.AluOpType
    # mmdt: B/reduction dtype; wdt: MLP dtype; tdt: transpose-path dtype
    cfg = {
        'fp32':   (f32, f32, f32),
        'fp32r':  (f32r, f32r, f32),
        'bf16':   (f32r, bf16, f32),
        'bf16b':  (bf16, bf16, bf16),
        'hybrid': (f16, f32r, f32r),
        'fp16':   (f16, f16, f16),
        'fp8':    (f8, f16, f16),
    }
    mmdt, wdt, tdt = cfg[precision]
    fp8_red = mmdt == f8          # DoubleRow fp8 reduction, 4 batches/pass
    QUADS = GROUP_B // 4          # 4 B quad-tiles (4 batches x 64 ants) per group
    RED_PASSES = QUADS if fp8_red else PAIRS
    if fp8_red:
        # every quad tile is 3968B/partition; all 16 fit in SBUF at once so
        # the B stream never stalls on pool slots
        b_bufs = GROUPS * QUADS
    else:
        b_bufs = 24 if mmdt in (bf16, f16) else 12   # B pair-tile slots
    # Matmul-feeding DRAM tensors are declared f32r directly (raw fp32 bits;
    # the PE truncates to f32r internally) so every load runs on the fast
    # hardware DGE instead of the gpsimd cast path.

    nc = bacc.Bacc()
    Bd = nc.dram_tensor('B', [BPC, NUM_M, NUM_USER, FEAT_B], mmdt, kind='ExternalInput')
    if fp8_red:
        # D packed host-side as [b-in-group, group, user, 2] so one tiny
        # contiguous DMA loads every group's D features
        Dtd = nc.dram_tensor('Dt', [GROUP_B, GROUPS, NUM_USER, 2], wdt,
                             kind='ExternalInput')
    else:
        Dtd = nc.dram_tensor('Dt', [2, NUM_USER, BPC], wdt, kind='ExternalInput')
    W1d = nc.dram_tensor('W1p', [64, 512], wdt, kind='ExternalInput')
    if fp8_red:
        # W2/W3/W4 pre-permuted host-side to the SBUF layout so their DMAs
        # are fully contiguous (the on-the-fly rearrange emitted hundreds of
        # tiny descriptors and hogged the DGE ring for ~7us)
        W2d = nc.dram_tensor('W2', [128, 4, 512], wdt, kind='ExternalInput')
        W3d = nc.dram_tensor('W3', [128, 4, 512], wdt, kind='ExternalInput')
        W4d = nc.dram_tensor('W4', [128, 4, 2], wdt, kind='ExternalInput')
    else:
        W2d = nc.dram_tensor('W2', [512, 512], wdt, kind='ExternalInput')
        W3d = nc.dram_tensor('W3', [512, 512], wdt, kind='ExternalInput')
        W4d = nc.dram_tensor('W4', [512, 2], wdt, kind='ExternalInput')
    BIAS123d = nc.dram_tensor('bias123', [128, 12], f32, kind='ExternalInput')
    B4d = nc.dram_tensor('b4', [2, 1], f32, kind='ExternalInput')
    Pd = nc.dram_tensor('P', [1, 1], f32, kind='ExternalInput')
    if fp8_red:
        OMd = nc.dram_tensor('omask', [128, QUADS, 2, 16], mmdt,
                             kind='ExternalInput')
    else:
        OMd = nc.dram_tensor('omask', [128, 16, 8], mmdt, kind='ExternalInput')
    I16d = nc.dram_tensor('ident16', [16, 16], tdt, kind='ExternalInput')
    Od = nc.dram_tensor('out', [2, NUM_USER, BPC], f32, kind='ExternalOutput')

    def wload(dst, src_ap):
        nc.scalar.dma_start(out=dst, in_=src_ap)

    with tile.TileContext(nc) as tc:
        with (
            tc.tile_pool(name='w', bufs=1) as wpool,
            tc.tile_pool(name='bt', bufs=b_bufs) as bpool,
            tc.tile_pool(name='bn', bufs=2) as nrm,
            tc.tile_pool(name='xp', bufs=2) as xpool,
            tc.tile_pool(name='hp', bufs=2) as hpool,
            tc.tile_pool(name='sp', bufs=2) as spool,
            tc.tile_pool(name='bnps', bufs=1, space='PSUM') as rp,
            tc.tile_pool(name='pxp', bufs=1, space='PSUM') as pt,
            tc.tile_pool(name='psh', bufs=3, space='PSUM') as ph,
        ):
            if fp8_red:
                # The PE clock ramps (0.65 -> 1.2 -> 2.4 GHz after ~3us of
                # continuous execution) and resets on every stall.  Burn the
                # ramp on throwaway matmuls (zero tile, no DMA dependency —
                # the memset is the vector ring's first op) while the first
                # B quad is still in flight, so real work starts near full
                # clock.
                warm = wpool.tile([128, 496], wdt)
                nc.vector.memset(warm[:], 0.0)
                wps = [rp.tile([16, 496], f32, name=f'bnps{q}_w', tag=f'bnps{q}')
                       for q in range(NCHUNK)]
                # ~5us of warmup: the PE hits full clock (~3us in) and all
                # of group 0's quads land before the first real matmul, so
                # the reduction runs gap-free at 2.4GHz instead of
                # trickling at DMA arrival with ramp resets
                for w in range(14):
                    nc.tensor.matmul(wps[w % NCHUNK][:], warm[:, 0:16], warm[:],
                                     start=True, stop=True)

            if fp8_red:
                omask = wpool.tile([128, QUADS, 2, 16], mmdt)
            else:
                omask = wpool.tile([128, 16, 8], mmdt)
            wload(omask, OMd[:])
            ident16 = wpool.tile([16, 16], tdt)
            nc.scalar.dma_start(out=ident16, in_=I16d[:])
            bias123 = wpool.tile([128, 12], f32)
            nc.scalar.dma_start(out=bias123, in_=BIAS123d[:])
            b4sb = wpool.tile([2, 1], f32)
            nc.scalar.dma_start(out=b4sb, in_=B4d[:])
            w1 = wpool.tile([64, 512], wdt)
            wload(w1, W1d[:])
            w2 = wpool.tile([128, 4, 512], wdt)
            w3 = wpool.tile([128, 4, 512], wdt)
            w4 = wpool.tile([128, 4, 2], wdt)
            if fp8_red:
                # w2/w4 ride the scalar ring after the small loads (done
                # well before the first L1 relu needs the ring); w3 is
                # slotted into the gpsimd ring between B quads below
                nc.scalar.dma_start(out=w2, in_=W2d[:])
                nc.scalar.dma_start(out=w4, in_=W4d[:])
                dsb = wpool.tile([GROUP_B, GROUPS, NUM_USER, 2], wdt)
                nc.scalar.dma_start(out=dsb, in_=Dtd[:])
            else:
                wload(w2, W2d[:].rearrange('(k p) m -> p k m', p=128))
                wload(w3, W3d[:].rearrange('(k p) m -> p k m', p=128))
                wload(w4, W4d[:].rearrange('(k p) c -> p k c', p=128))
            psb = wpool.tile([2, 1], f32)
            nc.scalar.dma_start(out=psb, in_=Pd[:].broadcast_to((2, 1)))

            # ---- emit all B loads up front; pool slots pace them ----
            all_bsrcs = []
            for g in range(GROUPS):
                bsrcs = []
                if fp8_red:
                    # group tile: 16 batches x 64 ants as [128, quad, 2, 1984];
                    # the DoubleRow matmul contracts partition x the "2" dim,
                    # so one pass reduces 4 batches.  Contraction element
                    # (p, i) of quad j holds batch 4j + 2i + p//64, antenna
                    # p%64 — encoded in the 4D source view.  A DGE ring only
                    # keeps ~1 DMA in flight, so quads alternate between the
                    # sync and gpsimd rings to pipeline issue with transfer.
                    # independent per-quad tiles keep the dependency
                    # granularity at one quad: a reduction pass never waits
                    # on a sibling quad's DMA
                    for j in range(QUADS):
                        bsrc = bpool.tile([128, 2, UF], mmdt)
                        b0 = g * GROUP_B + 4 * j
                        ring = nc.sync if (g * QUADS + j) % 2 == 0 else nc.gpsimd
                        ring.dma_start(
                            out=bsrc,
                            in_=Bd[b0:b0 + 4].rearrange(
                                '(i p1) a u f -> (p1 a) i (u f)', i=2, p1=2))
                        bsrcs.append(bsrc)
                    if g == 1:
                        # w3 slots in after group 1's quads; it arrives just
                        # ahead of group 0's L3 while later B quads are still
                        # far ahead of their consumption
                        nc.gpsimd.dma_start(out=w3, in_=W3d[:])
                else:
                    for j in range(PAIRS):
                        bsrc = bpool.tile([128, UF], mmdt)
                        b0 = g * GROUP_B + 2 * j
                        nc.sync.dma_start(
                            out=bsrc, in_=Bd[b0:b0 + 2].rearrange('b a u f -> b a (u f)'))
                        bsrcs.append(bsrc)
                all_bsrcs.append(bsrcs)

            # pass j's mask has ones only in its batches' columns, so all
            # passes of a group accumulate into the same PSUM chunks.
            bn_ps_of = {}
            bn64_of = {}

            def reduce_pair(g, j):
                # emit pass j's 4 chunk matmuls for group g's reduction
                if g >= GROUPS:
                    return
                if g not in bn_ps_of:
                    bn_ps_of[g] = [rp.tile([16, 496], f32, name=f'bnps{q}_{g}',
                                           tag=f'bnps{q}') for q in range(NCHUNK)]
                for q in range(NCHUNK):
                    if fp8_red:
                        nc.tensor.matmul(bn_ps_of[g][q][:], omask[:, j],
                                         all_bsrcs[g][j][:, :, ts(q, 496)],
                                         start=(j == 0), stop=(j == RED_PASSES - 1),
                                         perf_mode=mybir.MatmulPerfMode.DoubleRow)
                    else:
                        nc.tensor.matmul(bn_ps_of[g][q][:], omask[:, :, j],
                                         all_bsrcs[g][j][:, ts(q, 496)],
                                         start=(j == 0), stop=(j == RED_PASSES - 1))
                if j == RED_PASSES - 1:
                    bn64 = nrm.tile([16, NUM_USER, 64], tdt, name=f'bn64_{g}',
                                    tag='bn64')
                    for q in range(NCHUNK):
                        nc.vector.tensor_copy(bn64[:, 8 * q:8 * (q + 1), 0:FEAT_B],
                                              bn_ps_of[g][q][:])
                    if fp8_red:
                        # D features ride bn64's pad columns 62:64 so the PE
                        # transposes carry them straight into x^T rows 62:63
                        # (no per-group D DMA, no second writer on xT)
                        nc.vector.tensor_copy(bn64[:, :, FEAT_B:FEAT_B + 2],
                                              dsb[:, g])
                    bn64_of[g] = bn64

            # drip next group's reduce passes between MLP stages, matched to
            # the DMA arrival rate (RED_PASSES over ~12 MLP m-stages)
            stage_ctr = [0]
            pair_ctr = {}

            def drip(g):
                if g >= GROUPS:
                    return
                stage_ctr[0] += 1
                # all RED_PASSES emitted by stage 8 (end of L2): quad 3's
                # matmuls then fill the L2->L3 boundary (its PSUM casts
                # queue on the DVE after the L2 relus, before L3 needs them)
                target = min(RED_PASSES, (stage_ctr[0] * RED_PASSES + 7) // 8)
                while pair_ctr.get(g, 0) < target:
                    reduce_pair(g, pair_ctr.get(g, 0))
                    pair_ctr[g] = pair_ctr.get(g, 0) + 1

            for j in range(RED_PASSES):
                reduce_pair(0, j)

            # group g's sigmoid+normalize+store is emitted AFTER group g+1's
            # xT assembly, so the DVE queue services the next group's copies
            # (which gate the PE's L1) before the current group's epilogue.
            pending_norm = [None]

            for g in range(GROUPS):
                bsl = slice(g * GROUP_B, (g + 1) * GROUP_B)
                bn64 = bn64_of.pop(g)
                bn_ps_of.pop(g)

                # pre-drip two of the next group's reduction passes so the
                # PE has work queued while this group's PSUM casts + x^T
                # assembly run on the DVE (otherwise a ~5us PE hole)
                if fp8_red and g + 1 < GROUPS:
                    while pair_ctr.get(g + 1, 0) < 2:
                        reduce_pair(g + 1, pair_ctr.get(g + 1, 0))
                        pair_ctr[g + 1] = pair_ctr.get(g + 1, 0) + 1

                # ---- x^T [64 feats, 512 rows], row r = u*16 + b ----
                # Each PE transpose handles two users ([16,128] -> [128,16]);
                # user 2t lands on partitions 0:64, user 2t+1 on 64:128.
                xT = xpool.tile([64, ROWS_G], wdt)
                if not fp8_red:
                    # rows 62/63 of x^T are the D features; issued early on
                    # the scalar HWDGE ring
                    nc.scalar.dma_start(out=xT[62:64, :], in_=Dtd[:, :, bsl])
                px = pt.tile([128, 16 * (NUM_USER // 2)], tdt)
                for t in range(NUM_USER // 2):
                    nc.tensor.transpose(out=px[:, ts(t, GROUP_B)],
                                        in_=bn64[:, 2 * t:2 * t + 2, :],
                                        identity=ident16[:])
                xTv = xT[:].rearrange('p (t c) -> p t c', t=NUM_USER // 2, c=2 * GROUP_B)
                pxv = px[:].rearrange('p (t c) -> p t c', t=NUM_USER // 2, c=GROUP_B)
                if fp8_red:
                    # rows 0:62 are B_norm feats, 62:64 the D feats carried
                    # through the transpose via bn64's pad columns
                    nc.vector.tensor_copy(xTv[0:64, :, 0:GROUP_B], pxv[0:64])
                    nc.vector.tensor_copy(xTv[0:64, :, GROUP_B:2 * GROUP_B],
                                          pxv[64:128])
                else:
                    # copy only feat rows 0:62 (rows 62:127 of px are pad)
                    nc.vector.tensor_copy(xTv[0:62, :, 0:GROUP_B], pxv[0:62])
                    nc.vector.tensor_copy(xTv[0:62, :, GROUP_B:2 * GROUP_B],
                                          pxv[64:126])

                if pending_norm[0] is not None:
                    pending_norm[0]()
                    pending_norm[0] = None

                # ---- MLP, feature-major; next group's reduction matmuls are
                # dripped AFTER each stage's matmuls so a late B tile can
                # never stall ready MLP work (the PE executes in order) ----
                h1 = hpool.tile([128, 4, ROWS_G], wdt)
                for m in range(4):
                    ps = ph.tile([128, ROWS_G], f32, tag='ps')
                    nc.tensor.matmul(ps[:], w1[:, ts(m, 128)], xT[:],
                                     start=True, stop=True)
                    nc.scalar.activation(out=h1[:, m, :], in_=ps[:], func=AF.Relu,
                                         bias=bias123[:, 0 + m:1 + m], scale=1.0)
                    drip(g + 1)
                h2 = hpool.tile([128, 4, ROWS_G], wdt)
                for m in range(4):
                    ps = ph.tile([128, ROWS_G], f32, tag='ps')
                    for k in range(4):
                        nc.tensor.matmul(ps[:], w2[:, k, ts(m, 128)], h1[:, k, :],
                                         start=(k == 0), stop=(k == 3))
                    # bias+relu+cast on the DVE so the scalar engine (which
                    # also runs L1/L3 relu + sigmoid) stays off the critical
                    # path: out = max(ps + bias, 0)
                    nc.vector.tensor_scalar(
                        out=h2[:, m, :], in0=ps[:],
                        scalar1=bias123[:, 4 + m:5 + m], scalar2=0.0,
                        op0=ALU.add, op1=ALU.max)
                    drip(g + 1)
                h3 = hpool.tile([128, 4, ROWS_G], wdt)
                for m in range(4):
                    ps = ph.tile([128, ROWS_G], f32, tag='ps')
                    for k in range(4):
                        nc.tensor.matmul(ps[:], w3[:, k, ts(m, 128)], h2[:, k, :],
                                         start=(k == 0), stop=(k == 3))
                    # (gpsimd cannot read PSUM, so L3 shares the scalar
                    # engine with L1)
                    nc.scalar.activation(out=h3[:, m, :], in_=ps[:], func=AF.Relu,
                                         bias=bias123[:, 8 + m:9 + m], scale=1.0)
                    drip(g + 1)
                ps4 = ph.tile([2, ROWS_G], f32, tag='ps')
                for k in range(4):
                    nc.tensor.matmul(ps4[:], w4[:, k, :], h3[:, k, :],
                                     start=(k == 0), stop=(k == 3))

                def mk_norm(g=g, bsl=bsl, ps4=ps4):
                    def emit():
                        # ---- sigmoid + per-batch user-sum normalization ----
                        sg = spool.tile([2, NUM_USER, GROUP_B], f32,
                                        name=f'sg_{g}', tag='sg')
                        nc.scalar.activation(
                            out=sg[:],
                            in_=ps4[:].rearrange('c (u b) -> c u b', u=NUM_USER),
                            func=AF.Sigmoid, bias=b4sb[:], scale=1.0)
                        s2 = spool.tile([2, GROUP_B], f32, name=f's2_{g}', tag='s2')
                        nc.vector.tensor_reduce(
                            out=s2[:], in_=sg[:].rearrange('c u b -> c b u'),
                            axis=mybir.AxisListType.X, op=mybir.AluOpType.add)
                        rc = spool.tile([2, GROUP_B], f32, name=f'rc_{g}', tag='rc')
                        nc.vector.reciprocal(rc[:], s2[:])
                        nc.vector.tensor_scalar_mul(rc[:], rc[:], psb[:])
                        rbc = rc[:].unsqueeze(1).broadcast_to((2, NUM_USER, GROUP_B))
                        nc.vector.tensor_mul(sg[:], sg[:], rbc)
                        # stores ride the sync ring, idle once the B stream
                        # is done (the gpsimd ring's exit drain is slow, so
                        # keep it off the final store's critical path)
                        nc.sync.dma_start(out=Od[:, :, bsl], in_=sg[:])
                    return emit

                pending_norm[0] = mk_norm()
                stage_ctr[0] = 0
                while pair_ctr.get(g + 1, 0) < RED_PASSES:
                    reduce_pair(g + 1, pair_ctr.get(g + 1, 0))
                    pair_ctr[g + 1] = pair_ctr.get(g + 1, 0) + 1

            pending_norm[0]()

    nc.finalize()
    return nc


def _get_nc(precision):
    if precision not in _CACHE:
        _CACHE[precision] = _build(precision)
    return _CACHE[precision]


def _quantize_fp8_errdiff(B):
    """Cast B to fp8e4 with error diffusion along the antenna axis: each
    rounding residual is carried into the next antenna's value, so the
    64-way antenna sum of the quantized values matches the fp32 sum to
    ~1 fp8 ULP instead of accumulating sqrt(64) independent errors."""
    import ml_dtypes
    f8 = ml_dtypes.float8_e4m3
    Bt = np.ascontiguousarray(B.transpose(1, 0, 2, 3))   # [a, b, u, f]
    out = np.empty(Bt.shape, f8)
    err = np.zeros(Bt.shape[1:], np.float32)
    for a in range(Bt.shape[0]):
        v = Bt[a] + err
        q = v.astype(f8)
        err = v - q.astype(np.float32)
        out[a] = q
    return np.ascontiguousarray(out.transpose(1, 0, 2, 3))


def _prep_inputs(D, B, P_pow_normalized, W1, b1, W2, b2, W3, b3, W4, b4,
                 precision='fp32r'):
    import ml_dtypes
    f = np.float32
    wnp = {'bf16': ml_dtypes.bfloat16, 'bf16b': ml_dtypes.bfloat16,
           'fp16': np.float16, 'fp8': np.float16}.get(precision, np.float32)
    bnp = {'bf16b': ml_dtypes.bfloat16, 'hybrid': np.float16,
           'fp16': np.float16, 'fp8': ml_dtypes.float8_e4m3}.get(precision, np.float32)
    tnp = {'bf16b': ml_dtypes.bfloat16, 'fp16': np.float16,
           'fp8': np.float16}.get(precision, np.float32)
    D = np.asarray(D, f)
    B = np.ascontiguousarray(np.asarray(B, f))
    W1 = np.asarray(W1, f)
    # x^T rows are [B_norm(62), D(2)] while the reference x is [D(2), B_norm(62)]
    W1p = np.ascontiguousarray(np.concatenate([W1[2:64], W1[0:2]], axis=0))
    bias123 = np.empty((128, 12), f)
    for l, bb in enumerate((b1, b2, b3)):
        bb = np.asarray(bb, f)
        for m in range(4):
            bias123[:, 4 * l + m] = bb[128 * m:128 * (m + 1)]
    if precision == 'fp8':
        # quad mask: pass j covers batches 4j..4j+3; contraction element
        # (partition p, two-dim i) holds batch 4j + 2i + p//64, antenna p%64
        omask = np.zeros((128, 4, 2, 16), bnp)
        for j in range(4):
            omask[0:64, j, 0, 4 * j] = 1.0
            omask[64:128, j, 0, 4 * j + 1] = 1.0
            omask[0:64, j, 1, 4 * j + 2] = 1.0
            omask[64:128, j, 1, 4 * j + 3] = 1.0
    else:
        omask = np.zeros((128, 16, 8), bnp)
        for j in range(8):
            omask[0:64, 2 * j, j] = 1.0
            omask[64:128, 2 * j + 1, j] = 1.0
    if precision == 'fp8':
        # pre-permute to the SBUF [p, k, m] layout so device DMAs are
        # fully contiguous
        W2h = np.ascontiguousarray(
            np.asarray(W2, f).reshape(4, 128, 512).transpose(1, 0, 2))
        W3h = np.ascontiguousarray(
            np.asarray(W3, f).reshape(4, 128, 512).transpose(1, 0, 2))
        W4h = np.ascontiguousarray(
            np.asarray(W4, f).reshape(4, 128, 2).transpose(1, 0, 2))
    else:
        W2h = np.ascontiguousarray(np.asarray(W2, f))
        W3h = np.ascontiguousarray(np.asarray(W3, f))
        W4h = np.ascontiguousarray(np.asarray(W4, f))
    shared = {
        'W1p': W1p.astype(wnp),
        'W2': W2h.astype(wnp),
        'W3': W3h.astype(wnp),
        'W4': W4h.astype(wnp),
        'bias123': bias123,
        # omask dtype follows the reduction dtype
        'b4': np.asarray(b4, f).reshape(2, 1).copy(),
        'P': np.asarray(P_pow_normalized, f).reshape(1, 1).copy(),
        'omask': omask,
        'ident16': np.eye(16, dtype=f).astype(tnp),
    }
    if precision == 'fp8':
        Bq = _quantize_fp8_errdiff(B)
    in_maps = []
    for c in range(N_CORES):
        m = dict(shared)
        if precision == 'fp8':
            m['B'] = np.ascontiguousarray(Bq[c * BPC:(c + 1) * BPC])
            # D packed [b-in-group, group, user, 2] for one contiguous DMA
            m['Dt'] = np.ascontiguousarray(
                D[c * BPC:(c + 1) * BPC].reshape(4, 16, NUM_USER, 2)
                .transpose(1, 0, 2, 3)).astype(wnp)
        else:
            m['B'] = np.ascontiguousarray(B[c * BPC:(c + 1) * BPC]).astype(bnp)
            # D transposed host-side to [c, u, b] so its DMA is contiguous
            m['Dt'] = np.ascontiguousarray(
                D[c * BPC:(c + 1) * BPC].transpose(2, 1, 0)).astype(wnp)
        in_maps.append(m)
    return in_maps


def _run(inputs, trace=False, precision=None):
    from concourse.bass_utils import run_bass_kernel_spmd
    precision = precision or PRECISION
    nc = _get_nc(precision)
    in_maps = _prep_inputs(
        D=inputs['D'], B=inputs['B'], P_pow_normalized=inputs['P_pow_normalized'],
        W1=inputs['W1'], b1=inputs['b1'], W2=inputs['W2'], b2=inputs['b2'],
        W3=inputs['W3'], b3=inputs['b3'], W4=inputs['W4'], b4=inputs['b4'],
        precision=precision)
    res = run_bass_kernel_spmd(nc, in_maps, list(range(N_CORES)), trace=trace)
    # out is [2, u, b] per core -> [b, u, 2]
    out = np.concatenate(
        [res.results[c]['out'].transpose(2, 1, 0) for c in range(N_CORES)], axis=0)
    return np.ascontiguousarray(out, np.float32), res


def kernel(D, B, P_pow_normalized, D_0, W1, b1, W2, b2, W3, b3, W4, b4):
    out, _ = _run({'D': D, 'B': B, 'P_pow_normalized': P_pow_normalized,
                   'W1': W1, 'b1': b1, 'W2': W2, 'b2': b2, 'W3': W3, 'b3': b3,
                   'W4': W4, 'b4': b4})
    return out

